# revision 23
# baseline (speedup 1.0000x reference)
"""CAM graph layer (message passing + Linear + ELU) on 8 Trainium2 NeuronCores.

Reference computation (per batch b of N=21 joints, F=256 features):
    x_agg[b,i] = sum_j cam[i,j] * x[b,j]            (21x21 aggregation)
    y = ELU(concat([x_agg, x], -1) @ W.T + b)       (Linear 512->256)

Kernel restructure:
    P  = x @ [W1.T | W2.T]   (one 512-wide matmul; W1/W2 = halves of W)
    y[b,i] = sum_j cam[i,j]*P1[b,j] + P2[b,i] + bias
    ELU(y) = min(exp(y)-1, relu(y))

Per-core: 2048 batches = 43008 rows, processed in super-groups of
8064 rows (= lcm(128, 126)): cast-load fp32->fp16, xbar DMA-transpose to
feature-major, matmul groups of 126 rows (6 batches) with a block-diagonal
cam lhsT whose extra ones-row adds the bias, ELU, 1MB batched stores.
"""

import numpy as np

import concourse.bass as bass
import concourse.bacc as bacc
import concourse.mybir as mybir
import concourse.tile as tile
from concourse.bass_utils import run_bass_kernel_spmd

N_CORES = 8
N = 21
F = 256
OUT = 256
ROWS_PER_CORE = 43008          # 2048 batches * 21 joints
GROUP_BATCHES = 6
GROUP_ROWS = GROUP_BATCHES * N      # 126
SG_ROWS = 8064                 # lcm(128, 126): 63 load-chunks, 64 mm-groups
TAIL_SG_ROWS = 2688            # 43008 - 5*8064: 21 chunks, 21 groups + 42-row group
STORE_GROUPS = 8               # groups per output store DMA (1008 rows, ~1MB)
QUAD = 2                       # groups per PSUM tile (2 banks); batches ELU ops

f16 = mybir.dt.float16
f32 = mybir.dt.float32

_ALU = mybir.AluOpType
_ACT = mybir.ActivationFunctionType


def _emit_supergroup(nc, pools, consts, x_dram, y_dram, r0, rows, has_tail,
                     group_base):
    """Emit one super-group: rows [r0, r0+rows) of this core's shard."""
    n_chunks = rows // 128
    n_full = rows // GROUP_ROWS if not has_tail else (rows - 42) // GROUP_ROWS
    loadpool, tpool, psumpool, epool, rpool, ypool = pools
    wt0_sb, wt1_sb, cam6_sb, cam2_sb, p1rot, p1tail = consts

    # Cast-load fp32 -> fp16 (SWDGE), staged per (feature-half, chunk-half)
    # into INDEPENDENT tiles so the four load->transpose chains never
    # serialize on tile-granular WAR tracking.
    xT0 = tpool.tile([128, SG_ROWS], f16, tag="xT0")
    xT1 = tpool.tile([128, SG_ROWS], f16, tag="xT1")
    # Load path with NO SWDGE DMAs (SWDGE copies serialize globally against
    # xbar transposes): HWDGE fp32 loads -> GPSIMD fp32->fp16 cast ->
    # HWDGE xbar transpose, in slices of <=16 chunks for pipelining.
    c0 = 0
    si = 0
    while c0 < n_chunks:
        c1 = min(c0 + 16, n_chunks)
        nch = c1 - c0
        xc32 = loadpool.tile([128, 16, F], f32, tag="xc32", name="xc32")
        nc.scalar.dma_start(
            xc32[:, 0:nch, :],
            x_dram[r0 + c0 * 128 : r0 + c1 * 128, :].rearrange(
                "(c p) f -> p c f", p=128
            ),
        )
        for xT, f0, f1, fh in ((xT0, 0, 128, 0), (xT1, 128, 256, 1)):
            xc16 = loadpool.tile([128, 16 * 128], f16, tag=f"xc16_{fh}{si % 2}",
                                 name=f"xc16_{fh}{si % 2}")
            nc.gpsimd.tensor_copy(
                xc16[:, 0 : nch * 128].rearrange("p (c q) -> p c q", q=128),
                xc32[:, 0:nch, f0:f1],
            )
            nc.sync.dma_start_transpose(
                xT[:, c0 * 128 : c1 * 128].rearrange("p (c q) -> p c q", q=128),
                xc16[:, 0 : nch * 128],
            )
        c0 = c1
        si += 1

    # Quads: QUAD groups share one 4-bank PSUM tile so the post-matmul ops
    # (cast / exp / relu / combine) each process QUAD groups per instruction.
    # The caller software-pipelines: front() now, back() one quad later.
    quads = [(q0, min(QUAD, n_full - q0)) for q0 in range(0, n_full, QUAD)]
    if has_tail:
        quads.append((n_full, -1))  # sentinel: 42-row tail group

    for qi, (q0, qn) in enumerate(quads):
        is_tail = qn == -1
        mrows = 42 if is_tail else GROUP_ROWS
        nq = 1 if is_tail else qn
        p = psumpool.tile([128, QUAD, 512], f32, tag="psum")
        for qq in range(nq):
            gr0 = (q0 + qq) * GROUP_ROWS
            nc.tensor.matmul(
                p[0:mrows, qq, 0:512], xT0[:, gr0 : gr0 + mrows], wt0_sb[:, :],
                start=True, stop=False,
            )
            nc.tensor.matmul(
                p[0:mrows, qq, 0:512], xT1[:, gr0 : gr0 + mrows], wt1_sb[:, :],
                start=False, stop=True,
            )

        # P1 (cast to fp16) into a rotating rhs tile; partition GROUP_ROWS
        # (42 for the tail tile) holds the pre-written bias.
        p1t = p1tail if is_tail else p1rot[(group_base // QUAD + qi) % len(p1rot)]
        nc.scalar.copy(p1t[0:mrows, 0:nq, :], p[0:mrows, 0:nq, 0:OUT])

        yield dict(p=p, p1t=p1t, q0=q0, qn=nq, mrows=mrows, is_tail=is_tail,
                   r0=r0, y_dram=y_dram,
                   cam_sb=cam2_sb if is_tail else cam6_sb)


def _emit_back(nc, pools, st, flush):
    """Back half of one quad: cam matmul + ELU + (maybe) store flush."""
    loadpool, tpool, psumpool, epool, rpool, ypool = pools
    p, p1t, q0, qn, mrows = st["p"], st["p1t"], st["q0"], st["qn"], st["mrows"]
    cam_sb = st["cam_sb"]
    y_dram = st["y_dram"]

    for qq in range(qn):
        nc.tensor.matmul(
            p[0:mrows, qq, 256:512],
            cam_sb[0 : mrows + 1, 0:mrows],
            p1t[0 : mrows + 1, qq, :],
            start=False, stop=True, skip_group_check=True,
        )

    # ELU(y) = min(exp(y)-1, relu(y)), batched over the quad.
    esb = epool.tile([128, QUAD, OUT], f16, tag="esb")
    nc.scalar.activation(
        esb[0:mrows, 0:qn, :], p[0:mrows, 0:qn, 256:512], _ACT.Exp
    )
    rsb = rpool.tile([128, QUAD, OUT], f16, tag="rsb")
    nc.vector.tensor_scalar_max(
        rsb[0:mrows, 0:qn, :], p[0:mrows, 0:qn, 256:512], 0.0
    )

    if flush["ysb"] is None:
        flush["ysb"] = ypool.tile([128, STORE_GROUPS, OUT], f32, tag="ysb",
                                  name="ysb")
        flush["g0"] = q0
        flush["r0"] = st["r0"]
    ysb = flush["ysb"]
    slot = q0 - flush["g0"]
    nc.vector.scalar_tensor_tensor(
        ysb[0:mrows, slot : slot + qn, :],
        esb[0:mrows, 0:qn, :], 1.0, rsb[0:mrows, 0:qn, :],
        _ALU.subtract, _ALU.min,
    )
    if st["is_tail"]:
        # Store accumulated full groups, then the ragged 42-row group.
        if slot > 0:
            rf0 = flush["r0"] + flush["g0"] * GROUP_ROWS
            nc.scalar.dma_start(
                y_dram[rf0 : rf0 + slot * GROUP_ROWS, :].rearrange(
                    "(g p) f -> p g f", p=GROUP_ROWS
                ),
                ysb[0:GROUP_ROWS, 0:slot, :],
            )
        rt0 = flush["r0"] + q0 * GROUP_ROWS
        nc.scalar.dma_start(
            y_dram[rt0 : rt0 + 42, :], ysb[0:42, slot, :]
        )
        flush["ysb"] = None
    elif slot + qn == STORE_GROUPS:
        rf0 = flush["r0"] + flush["g0"] * GROUP_ROWS
        nc.scalar.dma_start(
            y_dram[rf0 : rf0 + (slot + qn) * GROUP_ROWS, :].rearrange(
                "(g p) f -> p g f", p=GROUP_ROWS
            ),
            ysb[0:GROUP_ROWS, 0 : slot + qn, :],
        )
        flush["ysb"] = None


def _build_nc():
    nc = bacc.Bacc("TRN2", target_bir_lowering=False, debug=False,
                   num_devices=N_CORES)
    x_dram = nc.dram_tensor("xs", [ROWS_PER_CORE, F], f32, kind="ExternalInput")
    wt_dram = nc.dram_tensor("wt", [F, 2 * OUT], f16, kind="ExternalInput")
    cam6_dram = nc.dram_tensor("cam6", [128, GROUP_ROWS], f16, kind="ExternalInput")
    cam2_dram = nc.dram_tensor("cam2", [128, 42], f16, kind="ExternalInput")
    bias_dram = nc.dram_tensor("biasr", [QUAD, OUT], f16, kind="ExternalInput")
    y_dram = nc.dram_tensor("y", [ROWS_PER_CORE, OUT], f32, kind="ExternalOutput")

    with tile.TileContext(nc) as tc:
        with (
            tc.tile_pool(name="consts", bufs=1) as cpool,
            tc.tile_pool(name="load", bufs=2) as loadpool,
            tc.tile_pool(name="xt", bufs=2) as tpool,
            tc.tile_pool(name="psum", bufs=4, space=bass.MemorySpace.PSUM) as psumpool,
            tc.tile_pool(name="e", bufs=3) as epool,
            tc.tile_pool(name="r", bufs=3) as rpool,
            tc.tile_pool(name="y", bufs=2) as ypool,
        ):
            wt0_sb = cpool.tile([128, 2 * OUT], f16, tag="wt0")
            wt1_sb = cpool.tile([128, 2 * OUT], f16, tag="wt1")
            cam6_sb = cpool.tile([128, GROUP_ROWS], f16, tag="cam6")
            cam2_sb = cpool.tile([128, 42], f16, tag="cam2")
            nc.sync.dma_start(wt0_sb[:, :], wt_dram[0:128, :])
            nc.sync.dma_start(wt1_sb[:, :], wt_dram[128:256, :])
            nc.sync.dma_start(cam6_sb[:, :], cam6_dram[:, :])
            nc.sync.dma_start(cam2_sb[:, :], cam2_dram[:, :])
            # Rotating cam-matmul rhs tiles; bias row (partition GROUP_ROWS /
            # 42 for the tail tile) is written once here and never again.
            p1rot = [cpool.tile([128, QUAD, OUT], f16, tag=f"p1rot{i}",
                                name=f"p1rot{i}")
                     for i in range(4)]
            p1tail = cpool.tile([128, QUAD, OUT], f16, tag="p1tail")
            for t in p1rot:
                nc.sync.dma_start(t[GROUP_ROWS : GROUP_ROWS + 1, :, :],
                                  bias_dram[:, :])
            nc.sync.dma_start(p1tail[42:43, 0:1, :], bias_dram[0:1, :])

            consts = (wt0_sb, wt1_sb, cam6_sb, cam2_sb, p1rot, p1tail)
            pools = (loadpool, tpool, psumpool, epool, rpool, ypool)

            # Software-pipelined by one quad: front(n) then back(n-1), so
            # each engine's queue always holds ready work while quad n's
            # cross-engine chain (copy -> cam matmul -> ELU) resolves.
            n_full_sg = ROWS_PER_CORE // SG_ROWS  # 5
            sgs = [(sg * SG_ROWS, SG_ROWS, False,
                    sg * (SG_ROWS // GROUP_ROWS))
                   for sg in range(n_full_sg)]
            sgs.append((n_full_sg * SG_ROWS, TAIL_SG_ROWS, True,
                        n_full_sg * (SG_ROWS // GROUP_ROWS)))
            flush = {"ysb": None}
            pending = None
            for (r0, rows, has_tail, gbase) in sgs:
                for st in _emit_supergroup(nc, pools, consts, x_dram, y_dram,
                                           r0, rows, has_tail=has_tail,
                                           group_base=gbase):
                    if pending is not None:
                        _emit_back(nc, pools, pending, flush)
                    pending = st
            _emit_back(nc, pools, pending, flush)

    nc.compile()
    return nc


_NC_CACHE = None


def _host_constants(cam, W, b):
    W = np.asarray(W, np.float32)
    cam = np.asarray(cam, np.float32)
    b = np.asarray(b, np.float32)
    # rhs of matmul1: [f, o2] with o2<256 -> W1.T, o2>=256 -> W2.T
    wt = np.concatenate([W[:, :F].T, W[:, F:].T], axis=1).astype(np.float16)
    # Block-diagonal cam.T (6 batches) + ones row for the bias term.
    cam6 = np.zeros((128, GROUP_ROWS), np.float32)
    for bb in range(GROUP_BATCHES):
        cam6[bb * N : (bb + 1) * N, bb * N : (bb + 1) * N] = cam.T
    cam6[GROUP_ROWS, :] = 1.0
    cam2 = np.zeros((128, 42), np.float32)
    for bb in range(2):
        cam2[bb * N : (bb + 1) * N, bb * N : (bb + 1) * N] = cam.T
    cam2[42, :] = 1.0
    biasr = np.tile(b.reshape(1, OUT), (QUAD, 1))
    return (wt, cam6.astype(np.float16), cam2.astype(np.float16),
            biasr.astype(np.float16))


def kernel(x, cam, W, b, n_joints):
    global _NC_CACHE
    x = np.ascontiguousarray(np.asarray(x, np.float32))
    assert x.shape == (N_CORES * ROWS_PER_CORE, F)
    wt, cam6, cam2, biasr = _host_constants(cam, W, b)

    if _NC_CACHE is None:
        _NC_CACHE = _build_nc()
    nc = _NC_CACHE

    in_maps = []
    for i in range(N_CORES):
        in_maps.append({
            "xs": x[i * ROWS_PER_CORE : (i + 1) * ROWS_PER_CORE, :],
            "wt": wt, "cam6": cam6, "cam2": cam2, "biasr": biasr,
        })
    res = run_bass_kernel_spmd(nc, in_maps, core_ids=list(range(N_CORES)))
    y = np.concatenate([res.results[i]["y"] for i in range(N_CORES)], axis=0)
    return y


# revision 24
# speedup vs baseline: 1.2182x; 1.2182x over previous
"""CAM graph layer (message passing + Linear + ELU) on 8 Trainium2 NeuronCores.

Reference computation (per batch b of N=21 joints, F=256 features):
    x_agg[b,i] = sum_j cam[i,j] * x[b,j]            (21x21 aggregation)
    y = ELU(concat([x_agg, x], -1) @ W.T + b)       (Linear 512->256)

Kernel restructure:
    P  = x @ [W1.T | W2.T]   (one 512-wide matmul; W1/W2 = halves of W)
    y[b,i] = sum_j cam[i,j]*P1[b,j] + P2[b,i] + bias
    ELU(y) = min(exp(y)-1, relu(y))

Per-core: 2048 batches = 43008 rows, processed in super-groups of
8064 rows (= lcm(128, 126)): cast-load fp32->fp16, xbar DMA-transpose to
feature-major, matmul groups of 126 rows (6 batches) with a block-diagonal
cam lhsT whose extra ones-row adds the bias, ELU, 1MB batched stores.
"""

import numpy as np

import concourse.bass as bass
import concourse.bacc as bacc
import concourse.mybir as mybir
import concourse.tile as tile
from concourse.bass_utils import run_bass_kernel_spmd

N_CORES = 8
N = 21
F = 256
OUT = 256
ROWS_PER_CORE = 43008          # 2048 batches * 21 joints
GROUP_BATCHES = 6
GROUP_ROWS = GROUP_BATCHES * N      # 126
SG_ROWS = 8064                 # lcm(128, 126): 63 load-chunks, 64 mm-groups
TAIL_SG_ROWS = 2688            # 43008 - 5*8064: 21 chunks, 21 groups + 42-row group
STORE_GROUPS = 8               # groups per output store DMA (1008 rows, ~1MB)
QUAD = 2                       # groups per PSUM tile (2 banks); batches ELU ops

f16 = mybir.dt.float16
f32 = mybir.dt.float32

_ALU = mybir.AluOpType
_ACT = mybir.ActivationFunctionType


def _emit_supergroup(nc, pools, consts, x_dram, y_dram, r0, rows, has_tail,
                     group_base):
    """Emit one super-group: rows [r0, r0+rows) of this core's shard."""
    n_chunks = rows // 128
    n_full = rows // GROUP_ROWS if not has_tail else (rows - 42) // GROUP_ROWS
    loadpool, tpool, psumpool, epool, rpool, ypool = pools
    wt0_sb, wt1_sb, cam6_sb, cam2_sb, p1rot, p1tail = consts

    # Cast-load fp32 -> fp16 (SWDGE), staged per (feature-half, chunk-half)
    # into INDEPENDENT tiles so the four load->transpose chains never
    # serialize on tile-granular WAR tracking.
    xT0 = tpool.tile([128, SG_ROWS], f16, tag="xT0")
    xT1 = tpool.tile([128, SG_ROWS], f16, tag="xT1")
    # SWDGE cast-loads (fp32->fp16 in the DMA), all four issued before any
    # xbar transpose. SWDGE copies and xbar transposes serialize globally
    # (mode-transition hazard), so explicit deps force every transpose
    # after ALL this SG's loads: one transition per super-group instead of
    # an alternating load->transpose staircase.
    halves = [(0, n_chunks // 2), (n_chunks // 2, n_chunks)]
    loads = []
    slices = []
    for hi, (c0, c1) in enumerate(halves):
        for xT, f0, f1, fh in ((xT0, 0, 128, 0), (xT1, 128, 256, 1)):
            xc = loadpool.tile([128, 4096], f16, tag=f"xc{fh}{hi}",
                               name=f"xc{fh}{hi}")
            nch = c1 - c0
            ld = nc.gpsimd.dma_start(
                xc[:, 0 : nch * 128].rearrange("p (c f) -> p c f", f=128),
                x_dram[r0 + c0 * 128 : r0 + c1 * 128, f0:f1].rearrange(
                    "(c p) f -> p c f", p=128
                ),
            )
            loads.append(ld)
            slices.append((xc, xT, c0, nch))
    for xc, xT, c0, nch in slices:
        tr = nc.sync.dma_start_transpose(
            xT[:, c0 * 128 : (c0 + nch) * 128].rearrange(
                "p (c q) -> p c q", q=128),
            xc[:, 0 : nch * 128],
        )
        for ld in loads:
            tile.add_dep_helper(tr.ins, ld.ins,
                                reason="xbar transpose after all SG loads")

    # Quads: QUAD groups share one 4-bank PSUM tile so the post-matmul ops
    # (cast / exp / relu / combine) each process QUAD groups per instruction.
    # The caller software-pipelines: front() now, back() one quad later.
    quads = [(q0, min(QUAD, n_full - q0)) for q0 in range(0, n_full, QUAD)]
    if has_tail:
        quads.append((n_full, -1))  # sentinel: 42-row tail group

    for qi, (q0, qn) in enumerate(quads):
        is_tail = qn == -1
        mrows = 42 if is_tail else GROUP_ROWS
        nq = 1 if is_tail else qn
        p = psumpool.tile([128, QUAD, 512], f32, tag="psum")
        for qq in range(nq):
            gr0 = (q0 + qq) * GROUP_ROWS
            nc.tensor.matmul(
                p[0:mrows, qq, 0:512], xT0[:, gr0 : gr0 + mrows], wt0_sb[:, :],
                start=True, stop=False,
            )
            nc.tensor.matmul(
                p[0:mrows, qq, 0:512], xT1[:, gr0 : gr0 + mrows], wt1_sb[:, :],
                start=False, stop=True,
            )

        # P1 (cast to fp16) into a rotating rhs tile; partition GROUP_ROWS
        # (42 for the tail tile) holds the pre-written bias.
        p1t = p1tail if is_tail else p1rot[(group_base // QUAD + qi) % len(p1rot)]
        nc.scalar.copy(p1t[0:mrows, 0:nq, :], p[0:mrows, 0:nq, 0:OUT])

        yield dict(p=p, p1t=p1t, q0=q0, qn=nq, mrows=mrows, is_tail=is_tail,
                   r0=r0, y_dram=y_dram,
                   cam_sb=cam2_sb if is_tail else cam6_sb)


def _emit_back(nc, pools, st, flush):
    """Back half of one quad: cam matmul + ELU + (maybe) store flush."""
    loadpool, tpool, psumpool, epool, rpool, ypool = pools
    p, p1t, q0, qn, mrows = st["p"], st["p1t"], st["q0"], st["qn"], st["mrows"]
    cam_sb = st["cam_sb"]
    y_dram = st["y_dram"]

    for qq in range(qn):
        nc.tensor.matmul(
            p[0:mrows, qq, 256:512],
            cam_sb[0 : mrows + 1, 0:mrows],
            p1t[0 : mrows + 1, qq, :],
            start=False, stop=True, skip_group_check=True,
        )

    # ELU(y) = min(exp(y)-1, relu(y)), batched over the quad.
    esb = epool.tile([128, QUAD, OUT], f16, tag="esb")
    nc.scalar.activation(
        esb[0:mrows, 0:qn, :], p[0:mrows, 0:qn, 256:512], _ACT.Exp
    )
    rsb = rpool.tile([128, QUAD, OUT], f16, tag="rsb")
    nc.vector.tensor_scalar_max(
        rsb[0:mrows, 0:qn, :], p[0:mrows, 0:qn, 256:512], 0.0
    )

    if flush["ysb"] is None:
        flush["ysb"] = ypool.tile([128, STORE_GROUPS, OUT], f32, tag="ysb",
                                  name="ysb")
        flush["g0"] = q0
        flush["r0"] = st["r0"]
    ysb = flush["ysb"]
    slot = q0 - flush["g0"]
    nc.vector.scalar_tensor_tensor(
        ysb[0:mrows, slot : slot + qn, :],
        esb[0:mrows, 0:qn, :], 1.0, rsb[0:mrows, 0:qn, :],
        _ALU.subtract, _ALU.min,
    )
    if st["is_tail"]:
        # Store accumulated full groups, then the ragged 42-row group.
        if slot > 0:
            rf0 = flush["r0"] + flush["g0"] * GROUP_ROWS
            nc.scalar.dma_start(
                y_dram[rf0 : rf0 + slot * GROUP_ROWS, :].rearrange(
                    "(g p) f -> p g f", p=GROUP_ROWS
                ),
                ysb[0:GROUP_ROWS, 0:slot, :],
            )
        rt0 = flush["r0"] + q0 * GROUP_ROWS
        nc.scalar.dma_start(
            y_dram[rt0 : rt0 + 42, :], ysb[0:42, slot, :]
        )
        flush["ysb"] = None
    elif slot + qn == STORE_GROUPS:
        rf0 = flush["r0"] + flush["g0"] * GROUP_ROWS
        nc.scalar.dma_start(
            y_dram[rf0 : rf0 + (slot + qn) * GROUP_ROWS, :].rearrange(
                "(g p) f -> p g f", p=GROUP_ROWS
            ),
            ysb[0:GROUP_ROWS, 0 : slot + qn, :],
        )
        flush["ysb"] = None


def _build_nc():
    nc = bacc.Bacc("TRN2", target_bir_lowering=False, debug=False,
                   num_devices=N_CORES)
    x_dram = nc.dram_tensor("xs", [ROWS_PER_CORE, F], f32, kind="ExternalInput")
    wt_dram = nc.dram_tensor("wt", [F, 2 * OUT], f16, kind="ExternalInput")
    cam6_dram = nc.dram_tensor("cam6", [128, GROUP_ROWS], f16, kind="ExternalInput")
    cam2_dram = nc.dram_tensor("cam2", [128, 42], f16, kind="ExternalInput")
    bias_dram = nc.dram_tensor("biasr", [QUAD, OUT], f16, kind="ExternalInput")
    y_dram = nc.dram_tensor("y", [ROWS_PER_CORE, OUT], f32, kind="ExternalOutput")

    with tile.TileContext(nc) as tc:
        with (
            tc.tile_pool(name="consts", bufs=1) as cpool,
            tc.tile_pool(name="load", bufs=2) as loadpool,
            tc.tile_pool(name="xt", bufs=2) as tpool,
            tc.tile_pool(name="psum", bufs=4, space=bass.MemorySpace.PSUM) as psumpool,
            tc.tile_pool(name="e", bufs=3) as epool,
            tc.tile_pool(name="r", bufs=3) as rpool,
            tc.tile_pool(name="y", bufs=2) as ypool,
        ):
            wt0_sb = cpool.tile([128, 2 * OUT], f16, tag="wt0")
            wt1_sb = cpool.tile([128, 2 * OUT], f16, tag="wt1")
            cam6_sb = cpool.tile([128, GROUP_ROWS], f16, tag="cam6")
            cam2_sb = cpool.tile([128, 42], f16, tag="cam2")
            nc.sync.dma_start(wt0_sb[:, :], wt_dram[0:128, :])
            nc.sync.dma_start(wt1_sb[:, :], wt_dram[128:256, :])
            nc.sync.dma_start(cam6_sb[:, :], cam6_dram[:, :])
            nc.sync.dma_start(cam2_sb[:, :], cam2_dram[:, :])
            # Rotating cam-matmul rhs tiles; bias row (partition GROUP_ROWS /
            # 42 for the tail tile) is written once here and never again.
            p1rot = [cpool.tile([128, QUAD, OUT], f16, tag=f"p1rot{i}",
                                name=f"p1rot{i}")
                     for i in range(4)]
            p1tail = cpool.tile([128, QUAD, OUT], f16, tag="p1tail")
            for t in p1rot:
                nc.sync.dma_start(t[GROUP_ROWS : GROUP_ROWS + 1, :, :],
                                  bias_dram[:, :])
            nc.sync.dma_start(p1tail[42:43, 0:1, :], bias_dram[0:1, :])

            consts = (wt0_sb, wt1_sb, cam6_sb, cam2_sb, p1rot, p1tail)
            pools = (loadpool, tpool, psumpool, epool, rpool, ypool)

            # Software-pipelined by one quad: front(n) then back(n-1), so
            # each engine's queue always holds ready work while quad n's
            # cross-engine chain (copy -> cam matmul -> ELU) resolves.
            n_full_sg = ROWS_PER_CORE // SG_ROWS  # 5
            sgs = [(sg * SG_ROWS, SG_ROWS, False,
                    sg * (SG_ROWS // GROUP_ROWS))
                   for sg in range(n_full_sg)]
            sgs.append((n_full_sg * SG_ROWS, TAIL_SG_ROWS, True,
                        n_full_sg * (SG_ROWS // GROUP_ROWS)))
            flush = {"ysb": None}
            pending = None
            for (r0, rows, has_tail, gbase) in sgs:
                for st in _emit_supergroup(nc, pools, consts, x_dram, y_dram,
                                           r0, rows, has_tail=has_tail,
                                           group_base=gbase):
                    if pending is not None:
                        _emit_back(nc, pools, pending, flush)
                    pending = st
            _emit_back(nc, pools, pending, flush)

    nc.compile()
    return nc


_NC_CACHE = None


def _host_constants(cam, W, b):
    W = np.asarray(W, np.float32)
    cam = np.asarray(cam, np.float32)
    b = np.asarray(b, np.float32)
    # rhs of matmul1: [f, o2] with o2<256 -> W1.T, o2>=256 -> W2.T
    wt = np.concatenate([W[:, :F].T, W[:, F:].T], axis=1).astype(np.float16)
    # Block-diagonal cam.T (6 batches) + ones row for the bias term.
    cam6 = np.zeros((128, GROUP_ROWS), np.float32)
    for bb in range(GROUP_BATCHES):
        cam6[bb * N : (bb + 1) * N, bb * N : (bb + 1) * N] = cam.T
    cam6[GROUP_ROWS, :] = 1.0
    cam2 = np.zeros((128, 42), np.float32)
    for bb in range(2):
        cam2[bb * N : (bb + 1) * N, bb * N : (bb + 1) * N] = cam.T
    cam2[42, :] = 1.0
    biasr = np.tile(b.reshape(1, OUT), (QUAD, 1))
    return (wt, cam6.astype(np.float16), cam2.astype(np.float16),
            biasr.astype(np.float16))


def kernel(x, cam, W, b, n_joints):
    global _NC_CACHE
    x = np.ascontiguousarray(np.asarray(x, np.float32))
    assert x.shape == (N_CORES * ROWS_PER_CORE, F)
    wt, cam6, cam2, biasr = _host_constants(cam, W, b)

    if _NC_CACHE is None:
        _NC_CACHE = _build_nc()
    nc = _NC_CACHE

    in_maps = []
    for i in range(N_CORES):
        in_maps.append({
            "xs": x[i * ROWS_PER_CORE : (i + 1) * ROWS_PER_CORE, :],
            "wt": wt, "cam6": cam6, "cam2": cam2, "biasr": biasr,
        })
    res = run_bass_kernel_spmd(nc, in_maps, core_ids=list(range(N_CORES)))
    y = np.concatenate([res.results[i]["y"] for i in range(N_CORES)], axis=0)
    return y


# revision 25
# speedup vs baseline: 1.3233x; 1.0863x over previous
"""CAM graph layer (message passing + Linear + ELU) on 8 Trainium2 NeuronCores.

Reference computation (per batch b of N=21 joints, F=256 features):
    x_agg[b,i] = sum_j cam[i,j] * x[b,j]            (21x21 aggregation)
    y = ELU(concat([x_agg, x], -1) @ W.T + b)       (Linear 512->256)

Kernel restructure:
    P  = x @ [W1.T | W2.T]   (one 512-wide matmul; W1/W2 = halves of W)
    y[b,i] = sum_j cam[i,j]*P1[b,j] + P2[b,i] + bias
    ELU(y) = min(exp(y)-1, relu(y))

Per-core: 2048 batches = 43008 rows, processed in super-groups of
8064 rows (= lcm(128, 126)): cast-load fp32->fp16, xbar DMA-transpose to
feature-major, matmul groups of 126 rows (6 batches) with a block-diagonal
cam lhsT whose extra ones-row adds the bias, ELU, 1MB batched stores.
"""

import numpy as np

import concourse.bass as bass
import concourse.bacc as bacc
import concourse.mybir as mybir
import concourse.tile as tile
from concourse.bass_utils import run_bass_kernel_spmd

N_CORES = 8
N = 21
F = 256
OUT = 256
ROWS_PER_CORE = 43008          # 2048 batches * 21 joints
GROUP_BATCHES = 6
GROUP_ROWS = GROUP_BATCHES * N      # 126
SG_ROWS = 8064                 # lcm(128, 126): 63 load-chunks, 64 mm-groups
TAIL_SG_ROWS = 2688            # 43008 - 5*8064: 21 chunks, 21 groups + 42-row group
STORE_GROUPS = 8               # groups per output store DMA (1008 rows, ~1MB)
QUAD = 2                       # groups per PSUM tile (2 banks); batches ELU ops

f16 = mybir.dt.float16
f32 = mybir.dt.float32

_ALU = mybir.AluOpType
_ACT = mybir.ActivationFunctionType


def _emit_supergroup(nc, pools, consts, x_dram, y_dram, r0, rows, has_tail,
                     group_base, xbar_state):
    """Emit one super-group: rows [r0, r0+rows) of this core's shard."""
    n_chunks = rows // 128
    n_full = rows // GROUP_ROWS if not has_tail else (rows - 42) // GROUP_ROWS
    loadpool, tpool, psumpool, epool, rpool, ypool = pools
    wt0_sb, wt1_sb, cam6_sb, cam2_sb, p1rot, p1tail = consts

    # Cast-load fp32 -> fp16 (SWDGE), staged per (feature-half, chunk-half)
    # into INDEPENDENT tiles so the four load->transpose chains never
    # serialize on tile-granular WAR tracking.
    xT0 = tpool.tile([128, SG_ROWS], f16, tag="xT0")
    xT1 = tpool.tile([128, SG_ROWS], f16, tag="xT1")
    # SWDGE cast-loads (fp32->fp16 in the DMA), all four issued before any
    # xbar transpose. SWDGE copies and xbar transposes serialize globally
    # (mode-transition hazard), so explicit deps force every transpose
    # after ALL this SG's loads: one transition per super-group instead of
    # an alternating load->transpose staircase.
    halves = [(0, n_chunks // 2), (n_chunks // 2, n_chunks)]
    loads = []
    slices = []
    for hi, (c0, c1) in enumerate(halves):
        for xT, f0, f1, fh in ((xT0, 0, 128, 0), (xT1, 128, 256, 1)):
            xc = loadpool.tile([128, 4096], f16, tag=f"xc{fh}{hi}",
                               name=f"xc{fh}{hi}")
            nch = c1 - c0
            ld = nc.gpsimd.dma_start(
                xc[:, 0 : nch * 128].rearrange("p (c f) -> p c f", f=128),
                x_dram[r0 + c0 * 128 : r0 + c1 * 128, f0:f1].rearrange(
                    "(c p) f -> p c f", p=128
                ),
            )
            for ptr in xbar_state["prev_trans"]:
                tile.add_dep_helper(ld.ins, ptr.ins,
                                    reason="SG loads after prev SG transposes")
            loads.append(ld)
            slices.append((xc, xT, c0, nch))
    trans = []
    for xc, xT, c0, nch in slices:
        tr = nc.sync.dma_start_transpose(
            xT[:, c0 * 128 : (c0 + nch) * 128].rearrange(
                "p (c q) -> p c q", q=128),
            xc[:, 0 : nch * 128],
        )
        for ld in loads:
            tile.add_dep_helper(tr.ins, ld.ins,
                                reason="xbar transpose after all SG loads")
        trans.append(tr)
    xbar_state["prev_trans"] = trans

    # Quads: QUAD groups share one 4-bank PSUM tile so the post-matmul ops
    # (cast / exp / relu / combine) each process QUAD groups per instruction.
    # The caller software-pipelines: front() now, back() one quad later.
    quads = [(q0, min(QUAD, n_full - q0)) for q0 in range(0, n_full, QUAD)]
    if has_tail:
        quads.append((n_full, -1))  # sentinel: 42-row tail group

    for qi, (q0, qn) in enumerate(quads):
        is_tail = qn == -1
        mrows = 42 if is_tail else GROUP_ROWS
        nq = 1 if is_tail else qn
        p = psumpool.tile([128, QUAD, 512], f32, tag="psum")
        for qq in range(nq):
            gr0 = (q0 + qq) * GROUP_ROWS
            nc.tensor.matmul(
                p[0:mrows, qq, 0:512], xT0[:, gr0 : gr0 + mrows], wt0_sb[:, :],
                start=True, stop=False,
            )
            nc.tensor.matmul(
                p[0:mrows, qq, 0:512], xT1[:, gr0 : gr0 + mrows], wt1_sb[:, :],
                start=False, stop=True,
            )

        # P1 (cast to fp16) into a rotating rhs tile; partition GROUP_ROWS
        # (42 for the tail tile) holds the pre-written bias.
        p1t = p1tail if is_tail else p1rot[(group_base // QUAD + qi) % len(p1rot)]
        nc.scalar.copy(p1t[0:mrows, 0:nq, :], p[0:mrows, 0:nq, 0:OUT])

        yield dict(p=p, p1t=p1t, q0=q0, qn=nq, mrows=mrows, is_tail=is_tail,
                   r0=r0, y_dram=y_dram,
                   cam_sb=cam2_sb if is_tail else cam6_sb)


def _emit_back(nc, pools, st, flush):
    """Back half of one quad: cam matmul + ELU + (maybe) store flush."""
    loadpool, tpool, psumpool, epool, rpool, ypool = pools
    p, p1t, q0, qn, mrows = st["p"], st["p1t"], st["q0"], st["qn"], st["mrows"]
    cam_sb = st["cam_sb"]
    y_dram = st["y_dram"]

    for qq in range(qn):
        nc.tensor.matmul(
            p[0:mrows, qq, 256:512],
            cam_sb[0 : mrows + 1, 0:mrows],
            p1t[0 : mrows + 1, qq, :],
            start=False, stop=True, skip_group_check=True,
        )

    # ELU(y) = min(exp(y)-1, relu(y)), batched over the quad.
    esb = epool.tile([128, QUAD, OUT], f16, tag="esb")
    nc.scalar.activation(
        esb[0:mrows, 0:qn, :], p[0:mrows, 0:qn, 256:512], _ACT.Exp
    )
    rsb = rpool.tile([128, QUAD, OUT], f16, tag="rsb")
    nc.vector.tensor_scalar_max(
        rsb[0:mrows, 0:qn, :], p[0:mrows, 0:qn, 256:512], 0.0
    )

    if flush["ysb"] is None:
        flush["ysb"] = ypool.tile([128, STORE_GROUPS, OUT], f32, tag="ysb",
                                  name="ysb")
        flush["g0"] = q0
        flush["r0"] = st["r0"]
    ysb = flush["ysb"]
    slot = q0 - flush["g0"]
    nc.vector.scalar_tensor_tensor(
        ysb[0:mrows, slot : slot + qn, :],
        esb[0:mrows, 0:qn, :], 1.0, rsb[0:mrows, 0:qn, :],
        _ALU.subtract, _ALU.min,
    )
    if st["is_tail"]:
        # Store accumulated full groups, then the ragged 42-row group.
        if slot > 0:
            rf0 = flush["r0"] + flush["g0"] * GROUP_ROWS
            nc.scalar.dma_start(
                y_dram[rf0 : rf0 + slot * GROUP_ROWS, :].rearrange(
                    "(g p) f -> p g f", p=GROUP_ROWS
                ),
                ysb[0:GROUP_ROWS, 0:slot, :],
            )
        rt0 = flush["r0"] + q0 * GROUP_ROWS
        nc.scalar.dma_start(
            y_dram[rt0 : rt0 + 42, :], ysb[0:42, slot, :]
        )
        flush["ysb"] = None
    elif slot + qn == STORE_GROUPS:
        rf0 = flush["r0"] + flush["g0"] * GROUP_ROWS
        nc.scalar.dma_start(
            y_dram[rf0 : rf0 + (slot + qn) * GROUP_ROWS, :].rearrange(
                "(g p) f -> p g f", p=GROUP_ROWS
            ),
            ysb[0:GROUP_ROWS, 0 : slot + qn, :],
        )
        flush["ysb"] = None


def _build_nc():
    nc = bacc.Bacc("TRN2", target_bir_lowering=False, debug=False,
                   num_devices=N_CORES)
    x_dram = nc.dram_tensor("xs", [ROWS_PER_CORE, F], f32, kind="ExternalInput")
    wt_dram = nc.dram_tensor("wt", [F, 2 * OUT], f16, kind="ExternalInput")
    cam6_dram = nc.dram_tensor("cam6", [128, GROUP_ROWS], f16, kind="ExternalInput")
    cam2_dram = nc.dram_tensor("cam2", [128, 42], f16, kind="ExternalInput")
    bias_dram = nc.dram_tensor("biasr", [QUAD, OUT], f16, kind="ExternalInput")
    y_dram = nc.dram_tensor("y", [ROWS_PER_CORE, OUT], f32, kind="ExternalOutput")

    with tile.TileContext(nc) as tc:
        with (
            tc.tile_pool(name="consts", bufs=1) as cpool,
            tc.tile_pool(name="load", bufs=2) as loadpool,
            tc.tile_pool(name="xt", bufs=2) as tpool,
            tc.tile_pool(name="psum", bufs=4, space=bass.MemorySpace.PSUM) as psumpool,
            tc.tile_pool(name="e", bufs=3) as epool,
            tc.tile_pool(name="r", bufs=3) as rpool,
            tc.tile_pool(name="y", bufs=2) as ypool,
        ):
            wt0_sb = cpool.tile([128, 2 * OUT], f16, tag="wt0")
            wt1_sb = cpool.tile([128, 2 * OUT], f16, tag="wt1")
            cam6_sb = cpool.tile([128, GROUP_ROWS], f16, tag="cam6")
            cam2_sb = cpool.tile([128, 42], f16, tag="cam2")
            nc.sync.dma_start(wt0_sb[:, :], wt_dram[0:128, :])
            nc.sync.dma_start(wt1_sb[:, :], wt_dram[128:256, :])
            nc.sync.dma_start(cam6_sb[:, :], cam6_dram[:, :])
            nc.sync.dma_start(cam2_sb[:, :], cam2_dram[:, :])
            # Rotating cam-matmul rhs tiles; bias row (partition GROUP_ROWS /
            # 42 for the tail tile) is written once here and never again.
            p1rot = [cpool.tile([128, QUAD, OUT], f16, tag=f"p1rot{i}",
                                name=f"p1rot{i}")
                     for i in range(4)]
            p1tail = cpool.tile([128, QUAD, OUT], f16, tag="p1tail")
            for t in p1rot:
                nc.sync.dma_start(t[GROUP_ROWS : GROUP_ROWS + 1, :, :],
                                  bias_dram[:, :])
            nc.sync.dma_start(p1tail[42:43, 0:1, :], bias_dram[0:1, :])

            consts = (wt0_sb, wt1_sb, cam6_sb, cam2_sb, p1rot, p1tail)
            pools = (loadpool, tpool, psumpool, epool, rpool, ypool)

            # Software-pipelined by one quad: front(n) then back(n-1), so
            # each engine's queue always holds ready work while quad n's
            # cross-engine chain (copy -> cam matmul -> ELU) resolves.
            n_full_sg = ROWS_PER_CORE // SG_ROWS  # 5
            sgs = [(sg * SG_ROWS, SG_ROWS, False,
                    sg * (SG_ROWS // GROUP_ROWS))
                   for sg in range(n_full_sg)]
            sgs.append((n_full_sg * SG_ROWS, TAIL_SG_ROWS, True,
                        n_full_sg * (SG_ROWS // GROUP_ROWS)))
            flush = {"ysb": None}
            xbar_state = {"prev_trans": []}
            pending = None
            for (r0, rows, has_tail, gbase) in sgs:
                for st in _emit_supergroup(nc, pools, consts, x_dram, y_dram,
                                           r0, rows, has_tail=has_tail,
                                           group_base=gbase,
                                           xbar_state=xbar_state):
                    if pending is not None:
                        _emit_back(nc, pools, pending, flush)
                    pending = st
            _emit_back(nc, pools, pending, flush)

    nc.compile()
    return nc


_NC_CACHE = None


def _host_constants(cam, W, b):
    W = np.asarray(W, np.float32)
    cam = np.asarray(cam, np.float32)
    b = np.asarray(b, np.float32)
    # rhs of matmul1: [f, o2] with o2<256 -> W1.T, o2>=256 -> W2.T
    wt = np.concatenate([W[:, :F].T, W[:, F:].T], axis=1).astype(np.float16)
    # Block-diagonal cam.T (6 batches) + ones row for the bias term.
    cam6 = np.zeros((128, GROUP_ROWS), np.float32)
    for bb in range(GROUP_BATCHES):
        cam6[bb * N : (bb + 1) * N, bb * N : (bb + 1) * N] = cam.T
    cam6[GROUP_ROWS, :] = 1.0
    cam2 = np.zeros((128, 42), np.float32)
    for bb in range(2):
        cam2[bb * N : (bb + 1) * N, bb * N : (bb + 1) * N] = cam.T
    cam2[42, :] = 1.0
    biasr = np.tile(b.reshape(1, OUT), (QUAD, 1))
    return (wt, cam6.astype(np.float16), cam2.astype(np.float16),
            biasr.astype(np.float16))


def kernel(x, cam, W, b, n_joints):
    global _NC_CACHE
    x = np.ascontiguousarray(np.asarray(x, np.float32))
    assert x.shape == (N_CORES * ROWS_PER_CORE, F)
    wt, cam6, cam2, biasr = _host_constants(cam, W, b)

    if _NC_CACHE is None:
        _NC_CACHE = _build_nc()
    nc = _NC_CACHE

    in_maps = []
    for i in range(N_CORES):
        in_maps.append({
            "xs": x[i * ROWS_PER_CORE : (i + 1) * ROWS_PER_CORE, :],
            "wt": wt, "cam6": cam6, "cam2": cam2, "biasr": biasr,
        })
    res = run_bass_kernel_spmd(nc, in_maps, core_ids=list(range(N_CORES)))
    y = np.concatenate([res.results[i]["y"] for i in range(N_CORES)], axis=0)
    return y
